# revision 1
# baseline (speedup 1.0000x reference)
"""Trainium2 Bass kernel for nn_BetterBot (tiny 2-layer transformer over
15-token streams, B=65536, D=8, H=2) — data-parallel over 8 NeuronCores.

Algebraic restructuring (validated vs reference in fp64 numpy):
  * Tokens live in a 32-value space t: 0-14 dice, 15-29 star, 30-31 btns.
    There is no positional encoding, so layer-1 q/k/v (and hence the exp'd
    score table P_h[t,t'] = exp(q1_h[t]·k1_h[t']/sqrt(hd))) depend only on
    token VALUES.  P is a 32x32 constant table computed from the weights.
  * 12 "slots" per batch element: 5 dice positions, 5 star positions, and
    the 2 possible btns VALUES (summon_lvl only has 2 values; the key-side
    contribution of the 5 btns positions is count-weighted by cb0/cb1 and
    the query-side btns rows are shared).
  * Host ships per-slot gathers of constant tables (P rows/cols, v1, E+bo)
    plus the btns counts; the device does all cross-token arithmetic:
    softmax-1 aggregation, the MLP/residual chain (PE matmuls in a
    feature-major layout via DMA-transpose crossings), full layer-2
    attention on the 12 slots, and the weighted mean + output projection.

Layouts: "B" = batch-on-partitions [128, ...] (products/softmax/etc),
"F" = feature-on-partitions [96=(slot,e), b] (PE matmuls).  DMA xbar
transposes move 128x96 fp16 blocks between them.
"""

import os
from contextlib import ExitStack

import numpy as np

import concourse.bass as bass
import concourse.bacc as bacc
import concourse.mybir as mybir
from concourse.bass_utils import run_bass_kernel_spmd
from concourse.tile import TileContext

F16 = mybir.dt.float16
F32 = mybir.dt.float32
AT = mybir.ActivationFunctionType
OP = mybir.AluOpType

NCORES = 8
B = 65536
BC = B // NCORES          # 8192 per core
G = 8                     # 128-row chunks per super-chunk
CH = 128
SCB = G * CH              # 1024 batch per super-chunk
NSC = BC // SCB           # 8 super-chunks per core
NS = 12                   # slots
H = 2
E8 = 8
FD = NS * E8              # 96 feature rows

_LAST_RESULTS = None      # BassKernelResults for test.py introspection


# --------------------------------------------------------------------------
# host-side preparation
# --------------------------------------------------------------------------

def _host_prep(inputs):
    f64 = lambda k: np.asarray(inputs[k], np.float64)
    dice = np.asarray(inputs['dice_type']).astype(np.int64)
    star = np.asarray(inputs['dice_star']).astype(np.int64)
    summ = np.asarray(inputs['summon_lvl']).astype(np.int64)

    E = np.concatenate([f64('emb_dice'), f64('emb_star'), f64('emb_btns')], 0)
    Wqkv0, bqkv0 = f64('Wqkv0'), f64('bqkv0')
    qkv1 = E @ Wqkv0.T + bqkv0
    q1, k1, v1 = qkv1[:, 0:8], qkv1[:, 8:16], qkv1[:, 16:24]
    P = np.zeros((H, 32, 32))
    for h in range(H):
        P[h] = np.exp(q1[:, 4*h:4*h+4] @ k1[:, 4*h:4*h+4].T * 0.5)
        P[h] *= 3000.0 / P[h].max()

    tok = np.concatenate([dice, 15 + star, 30 + summ], 1)          # [B,15]
    cnt30 = (summ == 0).sum(1).astype(np.float64)
    cb = np.stack([cnt30, 5.0 - cnt30], 1)                          # [B,2]
    ts = np.concatenate(
        [tok[:, 0:5], tok[:, 5:10],
         np.broadcast_to(np.array([30, 31]), (B, 2))], 1)           # [B,12]

    # p1 [B, (h,i,j)]  fp16
    p1 = P[:, ts[:, :, None], ts[:, None, :]]                       # [2,B,12,12]
    p1 = np.ascontiguousarray(p1.transpose(1, 0, 2, 3)).reshape(B, 288)
    p1 = p1.astype(np.float16)

    # v1t [B, (h,d5,j)] fp16  (d5=4 -> 1.0 for the denominator row)
    V5 = np.zeros((32, H, 5))
    for h in range(H):
        V5[:, h, 0:4] = v1[:, 4*h:4*h+4] / 8.0
        V5[:, h, 4] = 0.125
    v1t = V5[ts]                                                    # [B,12,2,5]
    v1t[:, 10, :, :] *= cb[:, 0, None, None]
    v1t[:, 11, :, :] *= cb[:, 1, None, None]
    v1t = np.ascontiguousarray(v1t.transpose(0, 2, 3, 1)).reshape(B, 120)
    v1t = v1t.astype(np.float16)

    # x0f [96, B] fp16 (F layout)
    xe = (E[ts] + f64('bo0')).reshape(B, FD)
    x0f = np.ascontiguousarray(xe.T).astype(np.float16)

    cb2 = cb.astype(np.float16)                                     # [B,2]

    def blockdiag(W):
        m = np.zeros((FD, FD))
        for i in range(NS):
            # out[(i,e'),b] = sum_e W[e',e] x[(i,e),b]  => lhsT[(i,e),(i,e')] = W[e',e]
            m[8*i:8*i+8, 8*i:8*i+8] = W.T
        return m.astype(np.float16)

    Wqkv1 = f64('Wqkv1')
    mats = {
        'wo0bd': blockdiag(f64('Wo0')),
        'wl0bd': blockdiag(f64('Wl0')),
        'wq2bd': blockdiag(Wqkv1[0:8]),
        'wk2bd': blockdiag(Wqkv1[8:16]),
        'wv2bd': blockdiag(Wqkv1[16:24]),
        'wo1bd': blockdiag(f64('Wo1')),
        'wl1bd': blockdiag(f64('Wl1')),
    }
    Wout = f64('Wout')
    woutu = np.zeros((FD, 32))
    for i in range(10):
        woutu[8*i:8*i+8, 0:20] = Wout.T / 15.0
    mats['woutu'] = woutu.astype(np.float16)
    woutv = np.zeros((FD, 64))
    for v in range(2):
        woutv[8*(10+v):8*(10+v)+8, 32*v:32*v+20] = Wout.T / 15.0
    mats['woutv'] = woutv.astype(np.float16)

    biasv = np.zeros((FD, 8), np.float32)
    bqkv1 = f64('bqkv1')
    for i in range(NS):
        biasv[8*i:8*i+8, 0] = f64('bl0')
        biasv[8*i:8*i+8, 1] = 0.5 * bqkv1[0:8]
        biasv[8*i:8*i+8, 2] = bqkv1[8:16]
        biasv[8*i:8*i+8, 3] = 0.125 * bqkv1[16:24]
        biasv[8*i:8*i+8, 4] = f64('bo1')
        biasv[8*i:8*i+8, 5] = f64('bl1')

    bout_c = np.broadcast_to(f64('bout').astype(np.float32), (128, 20)).copy()

    in_maps = []
    for c in range(NCORES):
        lo, hi = c * BC, (c + 1) * BC
        m = {
            'p1': p1[lo:hi],
            'v1t': v1t[lo:hi],
            'cb2': cb2[lo:hi],
            'x0f': np.ascontiguousarray(x0f[:, lo:hi]),
            'bout_c': bout_c,
            'biasv': biasv,
        }
        m.update(mats)
        in_maps.append(m)
    return in_maps


# --------------------------------------------------------------------------
# device kernel
# --------------------------------------------------------------------------

def _build_nc():
    nc = bacc.Bacc('TRN2', target_bir_lowering=False)

    d_p1 = nc.dram_tensor('p1', [BC, 288], F16, kind='ExternalInput')
    d_v1 = nc.dram_tensor('v1t', [BC, 120], F16, kind='ExternalInput')
    d_cb = nc.dram_tensor('cb2', [BC, 2], F16, kind='ExternalInput')
    d_x0 = nc.dram_tensor('x0f', [FD, BC], F16, kind='ExternalInput')
    d_boutc = nc.dram_tensor('bout_c', [128, 20], F32, kind='ExternalInput')
    d_biasv = nc.dram_tensor('biasv', [FD, 8], F32, kind='ExternalInput')
    d_mats = {}
    for nme, w in [('wo0bd', FD), ('wl0bd', FD), ('wq2bd', FD), ('wk2bd', FD),
                   ('wv2bd', FD), ('wo1bd', FD), ('wl1bd', FD),
                   ('woutu', 32), ('woutv', 64)]:
        d_mats[nme] = nc.dram_tensor(nme, [FD, w], F16, kind='ExternalInput')
    d_out = nc.dram_tensor('out', [BC, 20], F32, kind='ExternalOutput')

    # DRAM views grouped as [sc][p=128][g][w]
    def sc_view(d, w):
        return d[:, :].rearrange('(s g p) w -> s p g w', s=NSC, g=G, p=CH)

    v_p1 = sc_view(d_p1, 288)
    v_v1 = sc_view(d_v1, 120)
    v_cb = sc_view(d_cb, 2)
    v_out = sc_view(d_out, 20)

    ctx = ExitStack()
    with ctx:
        tc = ctx.enter_context(TileContext(nc))
        cpool = ctx.enter_context(tc.tile_pool(name='const', bufs=1))
        bpool = ctx.enter_context(tc.tile_pool(name='bside', bufs=2))
        wpool = ctx.enter_context(tc.tile_pool(name='work', bufs=1))
        fpool = ctx.enter_context(tc.tile_pool(name='fside', bufs=1))
        pspool = ctx.enter_context(tc.tile_pool(name='ps', bufs=2, space='PSUM'))
        psout = ctx.enter_context(tc.tile_pool(name='pso', bufs=1, space='PSUM'))

        # ---- constants ----
        t_x0 = cpool.tile([FD, BC], F16)
        nc.sync.dma_start(out=t_x0[:, :], in_=d_x0[:, :])
        t_biasv = cpool.tile([FD, 8], F32)
        nc.sync.dma_start(out=t_biasv[:, :], in_=d_biasv[:, :])
        t_boutc = cpool.tile([128, 20], F32)
        nc.sync.dma_start(out=t_boutc[:, :], in_=d_boutc[:, :])
        t_negb = cpool.tile([128, 1], F32)
        nc.vector.memset(t_negb[:, :], -3.5)
        t_m = {}
        for nme, w in [('wo0bd', FD), ('wl0bd', FD), ('wq2bd', FD),
                       ('wk2bd', FD), ('wv2bd', FD), ('wo1bd', FD),
                       ('wl1bd', FD), ('woutu', 32), ('woutv', 64)]:
            t_m[nme] = cpool.tile([FD, w], F16, name='t_' + nme, tag=nme)
            nc.sync.dma_start(out=t_m[nme][:, :], in_=d_mats[nme][:, :])

        def bias(col):
            return t_biasv[:, col:col + 1]

        def mm96(lhsT, rhs_ap, ps_tag, m=FD):
            """out[m,1024] = lhsT.T @ rhs (two N=512 matmuls)."""
            if m == FD:
                ps = pspool.tile([m, SCB], F32, tag='mm96',
                                 name='ps_' + ps_tag)
            else:
                ps = psout.tile([m, SCB], F32, tag=ps_tag,
                                name='ps_' + ps_tag)
            for nh in range(2):
                s = slice(nh * 512, nh * 512 + 512)
                nc.tensor.matmul(ps[:, s], lhsT[:, :], rhs_ap[:, s],
                                 start=True, stop=True)
            return ps

        for sc in range(NSC):
            # ---------------- DMA in ----------------
            tp1 = bpool.tile([CH, G, 288], F16, tag='tp1')
            nc.sync.dma_start(out=tp1[:, :, :], in_=v_p1[sc])
            tv1 = bpool.tile([CH, G, H, 5, NS], F16, tag='tv1')
            nc.sync.dma_start(
                out=tv1[:, :, :, :, :].rearrange('p g h d j -> p g (h d j)'),
                in_=v_v1[sc])
            tcb = bpool.tile([CH, G, 2], F16, tag='tcb')
            nc.sync.dma_start(out=tcb[:, :, :], in_=v_cb[sc])

            p1v = tp1[:, :, :].rearrange('p g (h i j) -> p g h i j',
                                         h=H, i=NS, j=NS)

            # ---------------- layer-1 aggregation (B layout) -------------
            # products p1 * v1t  -> [p, g, h, i, d5, j]; cb is pre-baked
            # into v1t's btns columns on the host.  Split per (h, d5) to
            # keep every operand <=3 free dims (ISA TENSOR3D limit).
            tpr = wpool.tile([CH, G, H, NS, 5, NS], F16, tag='prod')
            for h in range(H):
                for d5 in range(5):
                    eng = nc.vector if (h * 5 + d5) % 3 else nc.gpsimd
                    eng.tensor_tensor(
                        out=tpr[:, :, h, :, d5, :],
                        in0=p1v[:, :, h, :, :],
                        in1=tv1[:, :, h, d5, :].unsqueeze(2).broadcast_to(
                            [CH, G, NS, NS]),
                        op=OP.mult)

            def jreduce(src, tag):
                """sum over trailing j=12 of [p,g,h,i,d5,12] -> [p,g,h,i,d5]"""
                ta = wpool.tile([CH, G, H, NS, 5, 4], F16, tag=tag + 'a')
                nc.vector.tensor_tensor(out=ta, in0=src[:, :, :, :, :, 0:4],
                                        in1=src[:, :, :, :, :, 4:8], op=OP.add)
                nc.vector.tensor_tensor(out=ta, in0=ta,
                                        in1=src[:, :, :, :, :, 8:12], op=OP.add)
                tb = wpool.tile([CH, G, H, NS, 5, 2], F16, tag=tag + 'b')
                nc.vector.tensor_tensor(out=tb, in0=ta[:, :, :, :, :, 0:2],
                                        in1=ta[:, :, :, :, :, 2:4], op=OP.add)
                tn = wpool.tile([CH, G, H, NS, 5], F16, tag=tag + 'n')
                nc.vector.tensor_tensor(out=tn, in0=tb[:, :, :, :, :, 0],
                                        in1=tb[:, :, :, :, :, 1], op=OP.add)
                return tn

            tn1 = jreduce(tpr, 'r1')

            # o1 = num/den  -> [p, g, i, (h,d)]
            trd1 = wpool.tile([CH, G, H, NS], F32, tag='rd1')
            nc.vector.reciprocal(out=trd1, in_=tn1[:, :, :, :, 4])
            o1 = wpool.tile([CH, G, 128], F16, tag='o1')
            if sc == 0:
                nc.vector.memset(o1[:, :, 96:128], 0.0)
            o1v = o1[:, :, 0:96].rearrange('p g (i e) -> p g i e', e=E8) \
                .rearrange('p g i (h d) -> p g h i d', h=H)
            for h in range(H):
                nc.vector.tensor_tensor(
                    out=o1v[:, :, h], in0=tn1[:, :, h, :, 0:4],
                    in1=trd1[:, :, h, :].unsqueeze(3).broadcast_to(
                        [CH, G, NS, 4]),
                    op=OP.mult)

            # ---------------- cross to F, MLP chain on PE ----------------
            o1F = fpool.tile([128, SCB], F16, tag='o1F')
            for g in range(G):
                nc.sync.dma_start_transpose(
                    out=o1F[:, g * CH:(g + 1) * CH], in_=o1[:, g, :])

            xsl = t_x0[:, sc * SCB:(sc + 1) * SCB]
            psA = mm96(t_m['wo0bd'], o1F[0:FD, :], 'mmA')
            yF = fpool.tile([FD, SCB], F16, tag='yF')
            nc.vector.tensor_tensor(out=yF, in0=psA[:, :], in1=xsl, op=OP.add)

            psB = mm96(t_m['wl0bd'], yF, 'mmB')
            rF = fpool.tile([FD, SCB], F16, tag='rF')
            nc.scalar.activation(rF, psB[:, :], AT.Relu, bias=bias(0))
            x1F = fpool.tile([FD, SCB], F16, tag='x1F')
            nc.vector.tensor_tensor(out=x1F, in0=yF, in1=rF, op=OP.add)

            psQ = mm96(t_m['wq2bd'], x1F, 'mmQ')
            psK = mm96(t_m['wk2bd'], x1F, 'mmK')
            psV = mm96(t_m['wv2bd'], x1F, 'mmV')
            q2F = fpool.tile([128, SCB], F16, tag='q2F')
            if sc == 0:
                nc.vector.memset(q2F[FD:128, :], 0.0)
            nc.scalar.activation(q2F[0:FD, :], psQ[:, :], AT.Identity,
                                 bias=bias(1), scale=0.5)
            k2F = fpool.tile([128, SCB], F16, tag='k2F')
            if sc == 0:
                nc.vector.memset(k2F[FD:128, :], 0.0)
            nc.scalar.activation(k2F[0:FD, :], psK[:, :], AT.Identity,
                                 bias=bias(2))
            v2F = fpool.tile([128, SCB], F16, tag='v2F')
            if sc == 0:
                nc.vector.memset(v2F[FD:128, :], 0.0)
            nc.scalar.activation(v2F[0:FD, :], psV[:, :], AT.Identity,
                                 bias=bias(3), scale=0.125)

            # ---------------- cross back to B ----------------
            qB = bpool.tile([CH, G, 128], F16, tag='qB')
            kB = bpool.tile([CH, G, 128], F16, tag='kB')
            vB = bpool.tile([CH, G, 128], F16, tag='vB')
            for g in range(G):
                cs = slice(g * CH, (g + 1) * CH)
                nc.sync.dma_start_transpose(out=qB[:, g, :], in_=q2F[:, cs])
                nc.sync.dma_start_transpose(out=kB[:, g, :], in_=k2F[:, cs])
                nc.sync.dma_start_transpose(out=vB[:, g, :], in_=v2F[:, cs])

            # v2 re-layout to [p, g, h, d5, j] with ones in d5=4
            tv2 = bpool.tile([CH, G, H, 5, NS], F16, tag='tv2')
            vv = vB[:, :, 0:96].rearrange('p g (j e) -> p g j e', e=E8) \
                .rearrange('p g j (h d) -> p g h d j', h=H)
            for h in range(H):
                nc.gpsimd.tensor_copy(out=tv2[:, :, h, 0:4, :],
                                      in_=vv[:, :, h])
            nc.vector.memset(tv2[:, :, :, 4, :], 0.125)
            # key-side count weights on the two btns value slots
            for h in range(H):
                nc.vector.tensor_tensor(
                    out=tv2[:, :, h, :, 10:12], in0=tv2[:, :, h, :, 10:12],
                    in1=tcb[:, :, :].unsqueeze(2).broadcast_to([CH, G, 5, 2]),
                    op=OP.mult)

            # ---------------- layer-2 attention (B layout) ----------------
            tps = wpool.tile([CH, G, H, NS, NS, 4], F16, tag='sprod')
            qv = qB[:, :, 0:96].rearrange('p g (i e) -> p g i e', e=E8)
            kv = kB[:, :, 0:96].rearrange('p g (j e) -> p g j e', e=E8)
            for h in range(H):
                for g in range(G):
                    nc.vector.tensor_tensor(
                        out=tps[:, g, h],
                        in0=qv[:, g, :, 4*h:4*h+4].unsqueeze(2).broadcast_to(
                            [CH, NS, NS, 4]),
                        in1=kv[:, g, :, 4*h:4*h+4].unsqueeze(1).broadcast_to(
                            [CH, NS, NS, 4]),
                        op=OP.mult)
            # reduce over d=4
            tsd = wpool.tile([CH, G, H, NS, NS, 2], F16, tag='sd')
            nc.vector.tensor_tensor(out=tsd, in0=tps[:, :, :, :, :, 0:2],
                                    in1=tps[:, :, :, :, :, 2:4], op=OP.add)
            tsc = wpool.tile([CH, G, H, NS, NS], F16, tag='sc')
            nc.vector.tensor_tensor(out=tsc, in0=tsd[:, :, :, :, :, 0],
                                    in1=tsd[:, :, :, :, :, 1], op=OP.add)
            # shift scores by -3.5 inside exp (cancels in softmax) so fp16
            # never overflows on the real data range (s2 max ~13.4); clamp at
            # 14 is pure safety and does not fire on the reference inputs.
            nc.vector.tensor_scalar_min(out=tsc, in0=tsc, scalar1=14.0)
            tw = wpool.tile([CH, G, H, NS, NS], F16, tag='tw')
            nc.scalar.activation(tw, tsc, AT.Exp, bias=t_negb[:, 0:1])
            tpn = wpool.tile([CH, G, H, NS, 5, NS], F16, tag='prod')
            for h in range(H):
                for d5 in range(5):
                    eng = nc.vector if (h * 5 + d5) % 3 else nc.gpsimd
                    eng.tensor_tensor(
                        out=tpn[:, :, h, :, d5, :],
                        in0=tw[:, :, h, :, :],
                        in1=tv2[:, :, h, d5, :].unsqueeze(2).broadcast_to(
                            [CH, G, NS, NS]),
                        op=OP.mult)
            tn2 = jreduce(tpn, 'r2')
            trd2 = wpool.tile([CH, G, H, NS], F32, tag='rd2')
            nc.vector.reciprocal(out=trd2, in_=tn2[:, :, :, :, 4])
            o2 = wpool.tile([CH, G, 128], F16, tag='o2')
            if sc == 0:
                nc.vector.memset(o2[:, :, 96:128], 0.0)
            o2v = o2[:, :, 0:96].rearrange('p g (i e) -> p g i e', e=E8) \
                .rearrange('p g i (h d) -> p g h i d', h=H)
            for h in range(H):
                nc.vector.tensor_tensor(
                    out=o2v[:, :, h], in0=tn2[:, :, h, :, 0:4],
                    in1=trd2[:, :, h, :].unsqueeze(3).broadcast_to(
                        [CH, G, NS, 4]),
                    op=OP.mult)

            # ---------------- tail MLP + output ----------------
            o2F = fpool.tile([128, SCB], F16, tag='o2F')
            for g in range(G):
                nc.sync.dma_start_transpose(
                    out=o2F[:, g * CH:(g + 1) * CH], in_=o2[:, g, :])

            psF = mm96(t_m['wo1bd'], o2F[0:FD, :], 'mmF')
            zF = fpool.tile([FD, SCB], F16, tag='zF')
            nc.vector.scalar_tensor_tensor(
                out=zF, in0=psF[:, :], scalar=bias(4), in1=x1F,
                op0=OP.add, op1=OP.add)
            psG = mm96(t_m['wl1bd'], zF, 'mmG')
            r2F = fpool.tile([FD, SCB], F16, tag='r2F')
            nc.scalar.activation(r2F, psG[:, :], AT.Relu, bias=bias(5))
            x2F = fpool.tile([FD, SCB], F16, tag='x2F')
            nc.vector.tensor_tensor(out=x2F, in0=zF, in1=r2F, op=OP.add)

            psU = mm96(t_m['woutu'], x2F, 'mmU', m=32)
            psVw = mm96(t_m['woutv'], x2F, 'mmV2', m=64)
            uvS = fpool.tile([128, SCB], F16, tag='uvS')
            if sc == 0:
                nc.vector.memset(uvS[32:64, :], 0.0)
            nc.scalar.activation(uvS[0:32, :], psU[:, :], AT.Identity,
                                 bias=0.0)
            nc.vector.tensor_copy(out=uvS[64:128, :], in_=psVw[:, :])

            uvB = bpool.tile([CH, G, 128], F16, tag='uvB')
            for g in range(G):
                cs = slice(g * CH, (g + 1) * CH)
                nc.sync.dma_start_transpose(out=uvB[:, g, :], in_=uvS[:, cs])

            # out = u + cb0*v0 + cb1*v1 + bout
            t1 = wpool.tile([CH, G, 20], F16, tag='t1')
            nc.vector.tensor_tensor(
                out=t1, in0=uvB[:, :, 64:84],
                in1=tcb[:, :, 0:1].broadcast_to([CH, G, 20]), op=OP.mult)
            t2 = wpool.tile([CH, G, 20], F16, tag='t2')
            nc.vector.tensor_tensor(
                out=t2, in0=uvB[:, :, 96:116],
                in1=tcb[:, :, 1:2].broadcast_to([CH, G, 20]), op=OP.mult)
            nc.vector.tensor_tensor(out=t1, in0=t1, in1=t2, op=OP.add)
            nc.vector.tensor_tensor(out=t1, in0=t1, in1=uvB[:, :, 0:20],
                                    op=OP.add)
            tout = bpool.tile([CH, G, 20], F32, tag='tout')
            nc.vector.tensor_tensor(
                out=tout, in0=t1,
                in1=t_boutc[:, None, :].broadcast_to([CH, G, 20]), op=OP.add)
            nc.sync.dma_start(out=v_out[sc], in_=tout[:, :, :])

    nc.finalize()
    return nc


_NC_CACHE = None


def kernel(**inputs) -> np.ndarray:
    global _LAST_RESULTS, _NC_CACHE
    in_maps = _host_prep(inputs)
    if _NC_CACHE is None:
        _NC_CACHE = _build_nc()
    nc = _NC_CACHE
    trace = bool(int(os.environ.get('BETTERBOT_TRACE', '0')))
    res = run_bass_kernel_spmd(nc, in_maps, core_ids=list(range(NCORES)),
                               trace=trace)
    _LAST_RESULTS = res
    out = np.concatenate([r['out'] for r in res.results], 0)
    return out.astype(np.float32)

